# revision 19
# baseline (speedup 1.0000x reference)
"""Trainium2 Bass kernel for nn_CrossLayer (DCN-style cross stack).

Reference semantics (B=16384, D=1024, L=8):
    out_0 = x
    s_i = einsum('bd,d->b', out_i, W[i])
    out_{i+1} = x * s_i[:, None] + b[i] + x

Algebraic collapse: out_{i+1} = x * rho_{i+1} + b[i] with
    rho_1 = u_0 + 1,   rho_{l+1} = rho_l * u_l + c_l
    u_l[r] = <x[r, :], W[l]>          (U = x @ W.T, [B, L])
    c_l = <b[l-1], W[l]> + 1          (weights-only scalars)
    out = x * rho_8[:, None] + b[L-1]

Measured facts driving this version (vs the fp32-output baseline):
  - fp32 PE ops are dual-pass (LOW_HIGH): 128x128 transpose ~109ns,
    N=256 U-matmul ~216ns when streamed back-to-back; the PE stream is
    the critical resource (~31us warm), so stalls on it dominate.
  - The ut (PSUM->SBUF) copy on the scalar engine serialized ACT behind
    the U-matmuls and backed pst PSUM bufs up into the PE; it now runs
    on the vector engine.
  - y stored bf16 (halves write traffic; adds <= 2^-9 relative rounding,
    far under the 2e-2 gate), host upcasts.
  - rho_1..rho_8 in a single 8-step scan (c_0=1, init=1).

Sharding: data-parallel over batch; 8 cores x 2048 rows. Tiny (L, D)
weights replicated.
"""

import numpy as np

import concourse.bacc as bacc
import concourse.tile as tile
from concourse import mybir
from concourse.bass_utils import run_bass_kernel_spmd
from concourse.masks import make_identity

N_CORES = 8
B, D, L = 16384, 1024, 8
RPC = B // N_CORES          # rows per core (2048)
NT = RPC // 128             # 128-row tiles per core (16)
NP = NT // 2                # pairs per core (8)
NCH = D // 128              # 128-wide d chunks (8)
N_WARM = 16                 # bf16 warmup matmuls to lift HAM to K=8/8

LAST_RESULTS = None


def _build(cvals):
    """Trace + compile the per-core program. cvals = [c_1..c_{L-1}]."""
    nc = bacc.Bacc("TRN2", target_bir_lowering=False, debug=False)
    f32 = mybir.dt.float32
    bf16 = mybir.dt.bfloat16

    x_d = nc.dram_tensor("x", [RPC, D], f32, kind="ExternalInput")
    wt_d = nc.dram_tensor("wt", [128, NCH * L], f32, kind="ExternalInput")
    b7_d = nc.dram_tensor("b7r", [128, D], f32, kind="ExternalInput")
    y_d = nc.dram_tensor("y", [RPC, D], bf16, kind="ExternalOutput")

    # pair views: [h][p, t, d] with t in {0,1}
    x_pair = x_d.ap().rearrange("(h t p) d -> h p t d", t=2, p=128)
    x_tile = x_d.ap().rearrange("(t p) d -> t p d", p=128)
    y_tile = y_d.ap().rearrange("(t p) d -> t p d", p=128)

    with tile.TileContext(nc) as tc:
        with (
            tc.tile_pool(name="const", bufs=1) as cpool,
            tc.tile_pool(name="xp", bufs=5) as xpool,
            tc.tile_pool(name="xtp", bufs=3) as xtpool,
            tc.tile_pool(name="yp", bufs=3) as ypool,
            tc.tile_pool(name="small", bufs=6) as spool,
            tc.tile_pool(name="pst", bufs=3, space="PSUM") as pst,
            tc.tile_pool(name="psu", bufs=3, space="PSUM") as psu,
            tc.tile_pool(name="psr", bufs=2, space="PSUM") as psr,
        ):
            # --- first x data on the wire before anything else (pair 0 as
            # two tile loads so the first transposes can start earlier) ---
            xg0 = xpool.tile([128, 2, D], f32, tag="xg")
            nc.sync.dma_start(out=xg0[:, 0, :], in_=x_tile[0])
            nc.sync.dma_start(out=xg0[:, 1, :], in_=x_tile[1])
            xg1 = xpool.tile([128, 2, D], f32, tag="xg")
            nc.sync.dma_start(out=xg1[:], in_=x_pair[1])

            # --- warmup: dense bf16 matmuls to ramp HAM during DMA fill ---
            dummy = cpool.tile([128, 512], bf16)
            nc.gpsimd.memset(dummy[:], 0.0)
            for _ in range(N_WARM):
                pw = pst.tile([128, 512], f32, tag="pst")
                nc.tensor.matmul(pw[:], dummy[:, 0:128], dummy[:],
                                 start=True, stop=True)

            # --- constants ---
            ident = cpool.tile([128, 128], f32)
            make_identity(nc, ident[:])
            wt_sb = cpool.tile([128, NCH, L], f32)
            nc.sync.dma_start(out=wt_sb[:], in_=wt_d.ap().rearrange("p (c l) -> p c l", l=L))
            b7_sb = cpool.tile([128, D], f32)
            nc.sync.dma_start(out=b7_sb[:], in_=b7_d[:, :])
            # scan coefficients with c_0 = 1: rho_1..rho_8 from init 1.0
            c_sb = cpool.tile([128, L], f32)
            nc.gpsimd.memset(c_sb[:, 0:1], 1.0)
            for l in range(L - 1):
                nc.gpsimd.memset(c_sb[:, l + 1 : l + 2], cvals[l])
            ones = cpool.tile([128, 1], f32)
            nc.gpsimd.memset(ones[:], 1.0)

            for h in range(NP):
                if h == 0:
                    xg = xg0
                elif h == 1:
                    xg = xg1
                else:
                    xg = xpool.tile([128, 2, D], f32, tag="xg")
                    nc.sync.dma_start(out=xg[:], in_=x_pair[h])
                xg_c = xg[:].rearrange("p t (c d) -> p t c d", c=NCH)

                # transpose the pair's 16 chunks -> xT [128d, c, 256 rows]
                xT = xtpool.tile([128, NCH, 256], f32, tag="xT")
                for t in range(2):
                    for g in range(2):  # chunk half-groups of 4
                        pa = pst.tile([128, 4, 128], f32, tag="pst")
                        for j in range(4):
                            nc.tensor.transpose(
                                pa[:, j, :], xg_c[:, t, 4 * g + j, :], ident[:]
                            )
                        nc.scalar.copy(
                            xT[:, 4 * g : 4 * (g + 1), 128 * t : 128 * (t + 1)],
                            pa[:],
                        )

                # U^T for the pair: [L, 256] = sum_c wt_c.T @ xT_c
                ps_u = psu.tile([L, 256], f32, tag="psu")
                for c in range(NCH):
                    nc.tensor.matmul(
                        ps_u[:], wt_sb[:, c, :], xT[:, c, :],
                        start=(c == 0), stop=(c == NCH - 1),
                    )
                ut = spool.tile([L, 256], f32, tag="ut")
                nc.vector.tensor_copy(ut[:], ps_u[:])

                yt = ypool.tile([128, 2, D], bf16, tag="yt")
                for t in range(2):
                    # U tile back to row-partition orientation: [128, L]
                    pr = psr.tile([128, L], f32, tag="psr")
                    nc.tensor.transpose(
                        pr[:], ut[:, 128 * t : 128 * (t + 1)], ident[0:L, 0:L]
                    )
                    # rho_1..rho_8 in one scan: state=1; state = state*u_l + c_l
                    scano = spool.tile([128, L], f32, tag="scan")
                    nc.vector.tensor_tensor_scan(
                        scano[:], pr[:], c_sb[:], ones[:, 0:1],
                        mybir.AluOpType.mult, mybir.AluOpType.add,
                    )
                    # out = x * rho + b7  (bf16 store); per-tile write right
                    # after its STT so the t0 write overlaps the t1 STT and
                    # the final write tail is halved
                    nc.vector.scalar_tensor_tensor(
                        yt[:, t, :], xg[:, t, :], scano[:, L - 1 : L], b7_sb[:],
                        mybir.AluOpType.mult, mybir.AluOpType.add,
                    )
                    nc.gpsimd.dma_start(out=y_tile[2 * h + t], in_=yt[:, t, :])

    nc.compile()
    return nc


def kernel(x, W, b):
    global LAST_RESULTS
    x = np.ascontiguousarray(np.asarray(x), dtype=np.float32)
    W = np.ascontiguousarray(np.asarray(W), dtype=np.float32)
    b = np.ascontiguousarray(np.asarray(b), dtype=np.float32)
    assert x.shape == (B, D) and W.shape == (L, D) and b.shape == (L, D)

    cvals = [float(np.dot(b[l - 1].astype(np.float64), W[l].astype(np.float64)) + 1.0)
             for l in range(1, L)]
    wt = W.T.reshape(NCH, 128, L).transpose(1, 0, 2).reshape(128, NCH * L)
    wt = np.ascontiguousarray(wt, dtype=np.float32)
    b7r = np.ascontiguousarray(np.broadcast_to(b[L - 1], (128, D)), dtype=np.float32)

    nc = _build(cvals)

    shards = [x[i * RPC : (i + 1) * RPC] for i in range(N_CORES)]
    in_maps = [{"x": s, "wt": wt, "b7r": b7r} for s in shards]
    res = run_bass_kernel_spmd(nc, in_maps, core_ids=list(range(N_CORES)))
    LAST_RESULTS = res
    out = np.concatenate(
        [np.asarray(res.results[i]["y"]).astype(np.float32) for i in range(N_CORES)],
        axis=0,
    )
    return out


# revision 20
# speedup vs baseline: 1.0243x; 1.0243x over previous
"""Trainium2 Bass kernel for nn_CrossLayer (DCN-style cross stack).

Reference semantics (B=16384, D=1024, L=8):
    out_0 = x
    s_i = einsum('bd,d->b', out_i, W[i])
    out_{i+1} = x * s_i[:, None] + b[i] + x

Algebraic collapse: out_{i+1} = x * rho_{i+1} + b[i] with
    rho_1 = u_0 + 1,   rho_{l+1} = rho_l * u_l + c_l
    u_l[r] = <x[r, :], W[l]>          (U = x @ W.T, [B, L])
    c_l = <b[l-1], W[l]> + 1          (weights-only scalars)
    out = x * rho_8[:, None] + b[L-1]

Measured facts driving this version (vs the fp32-output baseline):
  - fp32 PE matmuls are emitted dual-instruction (LOW_HIGH weight halves)
    at ~4 cycles/moving-column: the 64 N=256 U-matmuls cost ~27us/core
    and the 128 transposes ~14us — the PE stream (~44-48us) is the
    critical resource, not DMA. bf16 alternatives for the U path were
    simulated on the real inputs and rejected: they inflate tail error
    metrics ~30-60x (fp16-x even fails p99 outright).
  - Exec time = ~9us fixed NEFF/DMA startup + ~4us first-tile transfer
    + PE stream + ~6us teardown. DMA transfers complete round-robin
    across everything in flight, so prefetch depth is kept moderate
    (deep prefetch delayed the first tile by 10us and regressed 13us).
  - The HAM PE clock climbs stepwise (~3.4us/window) and power-throttles
    to K=4/8 after ~48us of full-clock activity; the warmup burst starts
    the climb at t=0 inside the DMA fill window.
  - The ut (PSUM->SBUF) copy on the scalar engine serialized ACT behind
    the U-matmuls and backed pst PSUM bufs up into the PE; it now runs
    on the vector engine.
  - y stored bf16 (halves write traffic; adds <= 2^-9 relative rounding,
    ~10x under the 2e-2 gate on every metric), host upcasts. Per-tile
    writes issued right after each STT halve the final write tail.
  - rho_1..rho_8 in a single 8-step scan (c_0=1, init=1).

Sharding: data-parallel over batch; 8 cores x 2048 rows. Tiny (L, D)
weights replicated.
"""

import numpy as np

import concourse.bacc as bacc
import concourse.tile as tile
from concourse import mybir
from concourse.bass_utils import run_bass_kernel_spmd
from concourse.masks import make_identity

N_CORES = 8
B, D, L = 16384, 1024, 8
RPC = B // N_CORES          # rows per core (2048)
NT = RPC // 128             # 128-row tiles per core (16)
NP = NT // 2                # pairs per core (8)
NCH = D // 128              # 128-wide d chunks (8)
N_WARM = 16                 # bf16 warmup matmuls to lift HAM to K=8/8

LAST_RESULTS = None


def _build(cvals):
    """Trace + compile the per-core program. cvals = [c_1..c_{L-1}]."""
    nc = bacc.Bacc("TRN2", target_bir_lowering=False, debug=False)
    f32 = mybir.dt.float32
    bf16 = mybir.dt.bfloat16

    x_d = nc.dram_tensor("x", [RPC, D], f32, kind="ExternalInput")
    wt_d = nc.dram_tensor("wt", [128, NCH * L], f32, kind="ExternalInput")
    b7_d = nc.dram_tensor("b7r", [128, D], f32, kind="ExternalInput")
    y_d = nc.dram_tensor("y", [RPC, D], bf16, kind="ExternalOutput")

    # pair views: [h][p, t, d] with t in {0,1}
    x_pair = x_d.ap().rearrange("(h t p) d -> h p t d", t=2, p=128)
    x_tile = x_d.ap().rearrange("(t p) d -> t p d", p=128)
    y_tile = y_d.ap().rearrange("(t p) d -> t p d", p=128)

    with tile.TileContext(nc) as tc:
        with (
            tc.tile_pool(name="const", bufs=1) as cpool,
            tc.tile_pool(name="xp", bufs=5) as xpool,
            tc.tile_pool(name="xtp", bufs=3) as xtpool,
            tc.tile_pool(name="yp", bufs=3) as ypool,
            tc.tile_pool(name="small", bufs=6) as spool,
            tc.tile_pool(name="pst", bufs=3, space="PSUM") as pst,
            tc.tile_pool(name="psu", bufs=3, space="PSUM") as psu,
            tc.tile_pool(name="psr", bufs=2, space="PSUM") as psr,
        ):
            # --- first x data on the wire before anything else (pair 0 as
            # two tile loads so the first transposes can start earlier) ---
            xg0 = xpool.tile([128, 2, D], f32, tag="xg")
            nc.sync.dma_start(out=xg0[:, 0, :], in_=x_tile[0])
            nc.sync.dma_start(out=xg0[:, 1, :], in_=x_tile[1])
            xg1 = xpool.tile([128, 2, D], f32, tag="xg")
            nc.sync.dma_start(out=xg1[:], in_=x_pair[1])

            # --- warmup: dense bf16 matmuls to ramp HAM during DMA fill ---
            dummy = cpool.tile([128, 512], bf16)
            nc.gpsimd.memset(dummy[:], 0.0)
            for _ in range(N_WARM):
                pw = pst.tile([128, 512], f32, tag="pst")
                nc.tensor.matmul(pw[:], dummy[:, 0:128], dummy[:],
                                 start=True, stop=True)

            # --- constants ---
            ident = cpool.tile([128, 128], f32)
            make_identity(nc, ident[:])
            wt_sb = cpool.tile([128, NCH, L], f32)
            nc.sync.dma_start(out=wt_sb[:], in_=wt_d.ap().rearrange("p (c l) -> p c l", l=L))
            b7_sb = cpool.tile([128, D], f32)
            nc.sync.dma_start(out=b7_sb[:], in_=b7_d[:, :])
            # scan coefficients with c_0 = 1: rho_1..rho_8 from init 1.0
            c_sb = cpool.tile([128, L], f32)
            nc.gpsimd.memset(c_sb[:, 0:1], 1.0)
            for l in range(L - 1):
                nc.gpsimd.memset(c_sb[:, l + 1 : l + 2], cvals[l])
            ones = cpool.tile([128, 1], f32)
            nc.gpsimd.memset(ones[:], 1.0)

            for h in range(NP):
                if h == 0:
                    xg = xg0
                elif h == 1:
                    xg = xg1
                else:
                    xg = xpool.tile([128, 2, D], f32, tag="xg")
                    nc.sync.dma_start(out=xg[:], in_=x_pair[h])
                xg_c = xg[:].rearrange("p t (c d) -> p t c d", c=NCH)

                # transpose the pair's 16 chunks -> xT [128d, c, 256 rows]
                xT = xtpool.tile([128, NCH, 256], f32, tag="xT")
                for t in range(2):
                    for g in range(2):  # chunk half-groups of 4
                        pa = pst.tile([128, 4, 128], f32, tag="pst")
                        for j in range(4):
                            nc.tensor.transpose(
                                pa[:, j, :], xg_c[:, t, 4 * g + j, :], ident[:]
                            )
                        nc.scalar.copy(
                            xT[:, 4 * g : 4 * (g + 1), 128 * t : 128 * (t + 1)],
                            pa[:],
                        )

                # U^T for the pair: [L, 256] = sum_c wt_c.T @ xT_c
                ps_u = psu.tile([L, 256], f32, tag="psu")
                for c in range(NCH):
                    nc.tensor.matmul(
                        ps_u[:], wt_sb[:, c, :], xT[:, c, :],
                        start=(c == 0), stop=(c == NCH - 1),
                    )
                ut = spool.tile([L, 256], f32, tag="ut")
                nc.vector.tensor_copy(ut[:], ps_u[:])

                yt = ypool.tile([128, 2, D], bf16, tag="yt")
                for t in range(2):
                    # U tile back to row-partition orientation: [128, L]
                    pr = psr.tile([128, L], f32, tag="psr")
                    nc.tensor.transpose(
                        pr[:], ut[:, 128 * t : 128 * (t + 1)], ident[0:L, 0:L]
                    )
                    # rho_1..rho_8 in one scan: state=1; state = state*u_l + c_l
                    scano = spool.tile([128, L], f32, tag="scan")
                    nc.vector.tensor_tensor_scan(
                        scano[:], pr[:], c_sb[:], ones[:, 0:1],
                        mybir.AluOpType.mult, mybir.AluOpType.add,
                    )
                    # out = x * rho + b7  (bf16 store); per-tile write right
                    # after its STT so the t0 write overlaps the t1 STT and
                    # the final write tail is halved
                    nc.vector.scalar_tensor_tensor(
                        yt[:, t, :], xg[:, t, :], scano[:, L - 1 : L], b7_sb[:],
                        mybir.AluOpType.mult, mybir.AluOpType.add,
                    )
                    nc.gpsimd.dma_start(out=y_tile[2 * h + t], in_=yt[:, t, :])

    nc.compile()
    return nc


def kernel(x, W, b):
    global LAST_RESULTS
    x = np.ascontiguousarray(np.asarray(x), dtype=np.float32)
    W = np.ascontiguousarray(np.asarray(W), dtype=np.float32)
    b = np.ascontiguousarray(np.asarray(b), dtype=np.float32)
    assert x.shape == (B, D) and W.shape == (L, D) and b.shape == (L, D)

    cvals = [float(np.dot(b[l - 1].astype(np.float64), W[l].astype(np.float64)) + 1.0)
             for l in range(1, L)]
    wt = W.T.reshape(NCH, 128, L).transpose(1, 0, 2).reshape(128, NCH * L)
    wt = np.ascontiguousarray(wt, dtype=np.float32)
    b7r = np.ascontiguousarray(np.broadcast_to(b[L - 1], (128, D)), dtype=np.float32)

    nc = _build(cvals)

    shards = [x[i * RPC : (i + 1) * RPC] for i in range(N_CORES)]
    in_maps = [{"x": s, "wt": wt, "b7r": b7r} for s in shards]
    res = run_bass_kernel_spmd(nc, in_maps, core_ids=list(range(N_CORES)))
    LAST_RESULTS = res
    out = np.concatenate(
        [np.asarray(res.results[i]["y"]).astype(np.float32) for i in range(N_CORES)],
        axis=0,
    )
    return out
